# revision 29
# baseline (speedup 1.0000x reference)
"""Trainium2 Bass kernel for nn_Attention_38233798869191.

Full multi-head attention layer (B=2, S=2048, D=1024, H=16, dh=64) with the
reference's "faithful to original" reshape quirk, sharded over 8 NeuronCores
by splitting heads (tensor parallel): core c owns heads {2c, 2c+1}.

Per-core dataflow (everything transposed: feature dim on SBUF partitions):
  xT [1024, 4096]     (host-pretransposed x, shared by all cores)
  qT = (WqT_c.T @ xT) * SCALE   [128, 4096]   (2 heads x 64 dims)
  kT, vT likewise.
  v natural [sk, dh] built from vT via DMA XBAR transposes (2-byte dtype),
  with an extra ones-column so the p@v matmul also produces the softmax
  denominators.
  Per (b, head): scoresT[sk, sq] = kT.T @ qT; p = exp(scoresT) (no max
  subtraction -- scores are O(1) by construction); oT' = [v | 1].T @ p
  accumulated over sk chunks in PSUM.  The accumulator (and its denominator
  row) are XBAR-transposed back to token-major, divided by the denominator
  (per-partition scalars after the transpose), and written to a DRAM
  scratch as o natural [2048, 64].
  The reference's o.reshape(B, D, S).swapaxes trick means the output
  projection is y[b].T = Wo @ M[b] where M[b][h*64:(h+1)*64] is simply
  o_natural[b,h] reinterpreted as [64, 2048] (contiguous reshape), so the
  scratch is DMA'd back as [64, 2048] rows of M. Each core computes the
  partial y[b].T = Wo[:, c*128:(c+1)*128] @ M_c[b]; partials are summed on
  the host (the tensor-parallel all-reduce) and transposed back.

Scheduling: the attention inner loop is ACT-bound (exp of [128,1024] score
tiles at ~1.25us apiece vs ~1.04us of PE work per key chunk), while the
projection / output-projection phases are PE-bound with ACT idle. Engines
consume their queues in order, so emission order IS the schedule: batch-1
projection work is chopped into ~1-2us closures injected between attention
key-chunk iterations of batch 0's heads, outproj(0) closures likewise into
batch 1's heads, and the next rep's batch-0 projection interleaves with
outproj(1) at the tail. The PE queue stays fed during exp waits and the
TensorE does nothing but 512-column matmuls.
"""

import os
import sys

import numpy as np

for _p in ("/opt/trn_rl_repo", "/root/.axon_site/_ro/trn_rl_repo"):
    if os.path.isdir(_p) and _p not in sys.path:
        sys.path.insert(0, _p)

B, S, D, H, DH = 2, 2048, 1024, 16, 64
NSEQ = B * S  # 4096
SCALE = 1.0 / float(np.sqrt(DH))
N_CORES = 8
P = 128

DTYPE_MODE = os.environ.get("KERNEL_DTYPE_MODE", "bf16")


def _build_nc(mode, reps=1):
    import concourse.bass as bass  # noqa: F401
    import concourse.mybir as mybir
    import concourse.tile as tile
    from concourse import bacc

    assert mode == "bf16", "XBAR-transpose kernel requires bf16 tiles"
    f32 = mybir.dt.float32
    elt = mybir.dt.bfloat16
    mmdt = mybir.dt.bfloat16
    AF = mybir.ActivationFunctionType

    nc = bacc.Bacc(
        "TRN2",
        target_bir_lowering=False,
        debug=False,
        num_devices=N_CORES,
    )

    xT = nc.dram_tensor("xT", [D, NSEQ], elt, kind="ExternalInput")
    wqT = nc.dram_tensor("wqT", [D, P], elt, kind="ExternalInput")
    wkT = nc.dram_tensor("wkT", [D, P], elt, kind="ExternalInput")
    wvT = nc.dram_tensor("wvT", [D, P], elt, kind="ExternalInput")
    woT = nc.dram_tensor("woT", [P, D], elt, kind="ExternalInput")
    bqs = nc.dram_tensor("bqs", [P, 1], f32, kind="ExternalInput")  # raw bq
    bk = nc.dram_tensor("bk", [P, 1], f32, kind="ExternalInput")
    bv = nc.dram_tensor("bv", [P, 1], f32, kind="ExternalInput")
    # partial y, summed across cores on the host; bf16 partials are well
    # inside the error budget (partial entries ~1e-2, quant ~0.4%)
    ypT = nc.dram_tensor("ypT", [B, D, S], elt, kind="ExternalOutput")
    osc = nc.dram_tensor("osc", [2 * 2, S, DH], elt)  # o natural per (b, hl)

    # DRAM views
    # k global = ko*512 + ks*128 + p
    xTv = xT.ap().rearrange("(ko ks p) s -> ko p ks s", ks=4, p=P)
    oscM = osc.ap().rearrange("h (r k) d -> h r (k d)", r=64)  # [4, 64, 2048]

    def wview(w):
        return w.ap().rearrange("(kc p) m -> p kc m", p=P)  # [128, 8, 128]

    with tile.TileContext(nc) as tc:
        with (
            tc.tile_pool(name="persist", bufs=1) as pp,
            tc.tile_pool(name="xin", bufs=10) as xpool,
            # PSUM budget (8 banks):
            #   sps: 2 x [128,1024] f32 = 4 banks (scores + proj + outproj)
            #   ops: 2 x [65,1024] f32  = 4 banks (o accumulator, dbl-buf)
            tc.tile_pool(name="sps", bufs=2, space="PSUM") as sps,
            tc.tile_pool(name="ops", bufs=2, space="PSUM") as ops,
            tc.tile_pool(name="ptp", bufs=4) as ptp,
            tc.tile_pool(name="oup", bufs=2) as oup,
            tc.tile_pool(name="onp", bufs=2) as onp,
            tc.tile_pool(name="vtp", bufs=2) as vtp,
            tc.tile_pool(name="rcp", bufs=3) as rcp,
            tc.tile_pool(name="obp", bufs=2) as obp,
            tc.tile_pool(name="ysb", bufs=4) as ysbp,
        ):
            # persistent SBUF tensors
            w_sb = {}
            for name, w in (("q", wqT), ("k", wkT), ("v", wvT)):
                w_sb[name] = pp.tile([P, 8, P], elt, tag=f"w{name}", name=f"w{name}")
                nc.sync.dma_start(w_sb[name][:], wview(w))
            woT_sb = pp.tile([P, D], elt, tag="wo", name="wo")
            nc.sync.dma_start(woT_sb[:], woT.ap())
            bias_sb = {}
            for name, bt in (("q", bqs), ("k", bk), ("v", bv)):
                bias_sb[name] = pp.tile([P, 1], f32, tag=f"b{name}", name=f"b{name}")
                nc.sync.dma_start(bias_sb[name][:], bt.ap())
            # ACT-path eviction computes SCALE*acc + bias, so its q bias must
            # be pre-scaled (the DVE path computes (acc + raw_bias) * SCALE)
            bias_sc = dict(bias_sb)
            bias_sc["q"] = pp.tile([P, 1], f32, tag="bqsc", name="bqsc")
            nc.vector.tensor_scalar_mul(
                bias_sc["q"][:], bias_sb["q"][:], SCALE
            )
            qT_sb = pp.tile([P, NSEQ], elt, tag="qT", name="qT")
            kT_sb = pp.tile([P, NSEQ], elt, tag="kT", name="kT")
            vT_sb = pp.tile([P, NSEQ], elt, tag="vT", name="vT")
            proj_sb = {"q": qT_sb, "k": kT_sb, "v": vT_sb}
            M_sb = [pp.tile([P, S], elt, tag=f"M{b}", name=f"M{b}") for b in range(B)]
            v_nat = [pp.tile([P, 16, 72], elt, tag=f"vn{i}", name=f"vn{i}") for i in range(4)]
            ones_sb = pp.tile([P, 16], f32, tag="ones", name="ones")
            nc.vector.memset(ones_sb[:], 1.0)
            for bh in range(4):
                # ones column survives reps: the XBAR only rewrites [:, :, 0:64]
                nc.vector.tensor_copy(v_nat[bh][:, :, 64:65], ones_sb[:, :, None])


            def proj_closures(bi, evict):
                """Batch bi's q/k/v projection + v_nat build, chopped into
                ~1-2us closures for injection into attention. `evict`
                chooses the eviction engine: DVE when injected into
                exp-bound attention (ACT saturated), ACT when emitted into
                PE-dense stretches (DVE busy with output evictions)."""
                units = []
                xtiles = {}

                def dma_unit(sq, ko):
                    # dispatch from the Activation HWDGE queue: keeps the
                    # Sync queue short around the critical M reads
                    def run():
                        x_sb = xpool.tile([P, 4, 512], elt, tag="x", name="x")
                        for xh in range(2):
                            nc.scalar.dma_start(
                                x_sb[:, xh * 2 : (xh + 1) * 2, :],
                                xTv[
                                    ko,
                                    :,
                                    xh * 2 : (xh + 1) * 2,
                                    sq * 512 : (sq + 1) * 512,
                                ],
                            )
                        xtiles[(sq, ko)] = x_sb
                    return run

                def mm_unit(sq, n):
                    # one closure owns the PSUM accumulator's whole lifetime
                    # (8 matmuls + eviction): injected attention allocs from
                    # the same pool ring must never interleave with a live
                    # accumulator.
                    def run():
                        ps = sps.tile([P, 1024], f32, tag="s", name="s")
                        for ko in range(2):
                            for ks in range(4):
                                nc.tensor.matmul(
                                    ps[:, 0:512],
                                    w_sb[n][:, ko * 4 + ks, :].bitcast(mmdt),
                                    xtiles[(sq, ko)][:, ks, :].bitcast(mmdt),
                                    start=(ko == 0 and ks == 0),
                                    stop=(ko == 1 and ks == 3),
                                )
                        sl = slice(sq * 512, (sq + 1) * 512)
                        if evict == "dve":
                            # (acc + b) * s with per-partition bias AP
                            scale = SCALE if n == "q" else 1.0
                            nc.vector.tensor_scalar(
                                proj_sb[n][:, sl], ps[:, 0:512],
                                bias_sb[n][:], scale,
                                mybir.AluOpType.add, mybir.AluOpType.mult,
                            )
                        else:
                            kw = dict(scale=SCALE) if n == "q" else {}
                            nc.scalar.activation(
                                proj_sb[n][:, sl], ps[:, 0:512],
                                AF.Identity, bias=bias_sc[n][:], **kw,
                            )
                    return run

                def vnat_unit(hl):
                    # XBAR dst must be chunk-contiguous (strided chunk dst
                    # silently mis-strides at 16 chunks), so transpose into
                    # a contiguous temp and DVE-copy into the 72-wide
                    # ones-padded v_nat layout.
                    bh = bi * 2 + hl
                    hsl = slice(hl * 64, (hl + 1) * 64)

                    def run():
                        tmp = vtp.tile([P, 16, 64], elt, tag="vt", name="vt")
                        nc.sync.dma_start_transpose(
                            tmp[:], vT_sb[hsl, bi * S : (bi + 1) * S]
                        )
                        nc.vector.tensor_copy(v_nat[bh][:, :, 0:64], tmp[:])
                    return run

                # DMA units run ~one sq ahead of their consumers so the x
                # tiles are resident when the matmuls issue
                sqs = list(range(bi * 4, bi * 4 + 4))
                units.append(dma_unit(sqs[0], 0))
                units.append(dma_unit(sqs[0], 1))
                for i, sq in enumerate(sqs):
                    if i + 1 < len(sqs):
                        units.append(dma_unit(sqs[i + 1], 0))
                        units.append(dma_unit(sqs[i + 1], 1))
                    for n in "qkv":
                        units.append(mm_unit(sq, n))
                for hl in range(2):
                    units.append(vnat_unit(hl))
                return units

            _norm_ctr = [0]

            def normalize_half(bh, sqh, po):
                """XBAR the o accumulator (data rows + denominator row in one
                80-row tile, XBAR needs a multiple of 16) to token-major,
                divide by the per-partition denominators, store o natural to
                DRAM. No TensorE involvement."""
                obU = oup.tile([80, 1024], elt, tag="obU", name="obU")
                if _norm_ctr[0] < 2:
                    # pad rows 65..79 only once per pool slot: nothing else
                    # ever writes them, and the values are never consumed
                    nc.vector.memset(obU[64:80, :], 1.0)
                _norm_ctr[0] += 1
                nc.vector.tensor_copy(obU[0:65, :], po[:])
                onU = onp.tile([P, 8, 80], elt, tag="onU", name="onU")
                nc.sync.dma_start_transpose(onU[:], obU[:])
                rc = rcp.tile([P, 8, 1], f32, tag="rc", name="rc")
                nc.vector.reciprocal(rc[:], onU[:, :, 64:65])
                ob = obp.tile([P, 8, DH], elt, tag="ob", name="ob")
                for c in range(8):
                    nc.vector.tensor_scalar_mul(
                        ob[:, c, :], onU[:, c, 0:64], rc[:, c : c + 1, :]
                    )
                s0 = sqh * 1024
                nc.scalar.dma_start(
                    osc.ap()[bh, s0 : s0 + 1024, :].rearrange(
                        "(t p) d -> p t d", p=P
                    ),
                    ob[:],
                )

            def attention_head(b, hl, extra=None, period=3):
                bh = b * 2 + hl
                hsl = slice(hl * 64, (hl + 1) * 64)
                it = 0
                for sqh in range(2):  # halves of 1024 queries
                    sq0 = b * S + sqh * 1024
                    po = ops.tile([65, 1024], f32, tag="oacc", name="oacc")

                    def emit_pv(kc, ptile, po=po, bh=bh):
                        for half in range(2):
                            nc.tensor.matmul(
                                po[:, half * 512 : (half + 1) * 512],
                                v_nat[bh][:, kc, 0:65].bitcast(mmdt),
                                ptile[
                                    :, half * 512 : (half + 1) * 512
                                ].bitcast(mmdt),
                                start=(kc == 0),
                                stop=(kc == 15),
                            )

                    # software-pipelined: pv lags two kc so the next qk runs
                    # on PE while ACT does exp
                    pending_pv = []
                    for kc in range(16):
                        k0 = b * S + kc * P
                        ps2 = sps.tile([P, 1024], f32, tag="s", name="s")
                        for half in range(2):
                            nc.tensor.matmul(
                                ps2[:, half * 512 : (half + 1) * 512],
                                kT_sb[hsl, k0 : k0 + P].bitcast(mmdt),
                                qT_sb[
                                    hsl,
                                    sq0 + half * 512 : sq0 + (half + 1) * 512,
                                ].bitcast(mmdt),
                                start=True,
                                stop=True,
                            )
                        ptile = ptp.tile([P, 1024], elt, tag="pt", name="pt")
                        nc.scalar.activation(ptile[:], ps2[:], AF.Exp)
                        pending_pv.append((kc, ptile))
                        if len(pending_pv) > 2:
                            emit_pv(*pending_pv.pop(0))
                        if extra and it % period == period - 1:
                            extra.pop(0)()
                        it += 1
                    for args in pending_pv:
                        emit_pv(*args)
                        # keep feeding the PE while the pv flush serially
                        # waits on the trailing exps
                        if extra:
                            extra.pop(0)()

                    normalize_half(bh, sqh, po)
                # M rows for this head
                nc.sync.dma_start(
                    M_sb[b][hsl.start + 0 : hsl.start + 64, :],
                    oscM[bh],
                )

            _ysb_live = {}

            def outproj_unit(b, mo, nh, evict):
                def run():
                    if nh == 0:
                        _ysb_live[(b, mo)] = ysbp.tile(
                            [P, 2 * 1024], elt, tag="y", name="y"
                        )
                    ysb = _ysb_live[(b, mo)]
                    py = sps.tile([P, 1024], f32, tag="s", name="s")
                    for half in range(2):
                        n0 = nh * 1024 + half * 512
                        nc.tensor.matmul(
                            py[:, half * 512 : (half + 1) * 512],
                            woT_sb[:, mo * P : (mo + 1) * P].bitcast(mmdt),
                            M_sb[b][:, n0 : n0 + 512].bitcast(mmdt),
                            start=True,
                            stop=True,
                        )
                    dst = ysb[:, nh * 1024 : (nh + 1) * 1024]
                    if evict == "dve" or (evict == "alt" and nh == 0):
                        nc.vector.tensor_copy(dst, py[:])
                    else:
                        nc.scalar.copy(dst, py[:])
                    if nh == 1:
                        # y writes wait on long PE->DVE chains; dispatch from
                        # GpSimd to keep them off the Sync queue
                        nc.gpsimd.dma_start(
                            ypT.ap()[b, mo * P : (mo + 1) * P, :], ysb[:]
                        )
                return run

            def outproj_units(b, evict):
                return [
                    outproj_unit(b, mo, nh, evict)
                    for mo in range(8)
                    for nh in range(2)
                ]

            def drain(units):
                while units:
                    units.pop(0)()

            # emission order is the schedule (engines dequeue in order).
            # Per rep: batch-0 attention absorbs the PREVIOUS rep's
            # outproj(1) units plus batch-1 projection closures; batch-1
            # attention absorbs outproj(0) plus the NEXT rep's batch-0
            # projection. Almost everything off the attention critical path
            # runs inside an exp-bound attention stretch, so the PE queue
            # never drains and no engine idles for long.
            # outproj units depend on M reads that clear the Sync queue a
            # few us after the producing head finishes, so they sit AFTER
            # the projection closures in each extra list — by the time the
            # popping reaches them, their M rows are resident.
            ex0 = proj_closures(0, "act")
            op1_prev = []
            for rep in range(reps):
                drain(ex0)
                exA = proj_closures(1, "dve") + op1_prev
                attention_head(0, 0, extra=exA, period=2)
                attention_head(0, 1, extra=exA, period=2)
                drain(exA)
                ex0 = proj_closures(0, "dve") if rep + 1 < reps else []
                exB = ex0 + outproj_units(0, "dve")
                attention_head(1, 0, extra=exB, period=2)
                attention_head(1, 1, extra=exB, period=2)
                drain(exB)
                ex0 = []
                op1_prev = outproj_units(1, "dve")
            drain(op1_prev)

    nc.compile()
    return nc


_CACHE = {}


def _np_elt(mode):
    import ml_dtypes

    return ml_dtypes.bfloat16


def _get_runner(mode, reps=1):
    """Build (once) the compiled kernel + a persistent jitted executor."""
    key = (mode, reps)
    if key in _CACHE:
        return _CACHE[key]

    import jax
    import jax.numpy as jnp  # noqa: F401
    from jax.sharding import Mesh, PartitionSpec
    from jax.experimental.shard_map import shard_map
    import concourse.mybir as mybir
    from concourse import bass2jax

    nc = _build_nc(mode, reps)
    bass2jax.install_neuronx_cc_hook()

    partition_name = (
        nc.partition_id_tensor.name if nc.partition_id_tensor else None
    )
    in_names = []
    out_names = []
    out_avals = []
    for alloc in nc.m.functions[0].allocations:
        if not isinstance(alloc, mybir.MemoryLocationSet):
            continue
        name = alloc.memorylocations[0].name
        if alloc.kind == "ExternalInput":
            if name != partition_name:
                in_names.append(name)
        elif alloc.kind == "ExternalOutput":
            out_names.append(name)
            shape = tuple(alloc.tensor_shape)
            dtype = mybir.dt.np(alloc.dtype)
            out_avals.append(jax.core.ShapedArray(shape, dtype))
    n_params = len(in_names)
    n_outs = len(out_avals)
    all_in_names = list(in_names) + list(out_names)
    if partition_name is not None:
        all_in_names.append(partition_name)
    all_in_names = tuple(all_in_names)

    def _body(*args):
        operands = list(args)
        if partition_name is not None:
            operands.append(bass2jax.partition_id_tensor())
        outs = bass2jax._bass_exec_p.bind(
            *operands,
            out_avals=tuple(out_avals),
            in_names=all_in_names,
            out_names=tuple(out_names),
            lowering_input_output_aliases=(),
            sim_require_finite=True,
            sim_require_nnan=True,
            nc=nc,
        )
        return tuple(outs)

    devices = jax.devices()[:N_CORES]
    mesh = Mesh(np.asarray(devices), ("core",))
    in_specs = (PartitionSpec("core"),) * (n_params + n_outs)
    out_specs = (PartitionSpec("core"),) * n_outs
    donate = tuple(range(n_params, n_params + n_outs))
    sharded = jax.jit(
        shard_map(
            _body, mesh=mesh, in_specs=in_specs, out_specs=out_specs,
            check_rep=False,
        ),
        donate_argnums=donate,
        keep_unused=True,
    )

    zero_out_shapes = [
        ((N_CORES * a.shape[0],) + tuple(a.shape[1:]), a.dtype)
        for a in out_avals
    ]

    def execute(in_maps):
        concat_in = [
            np.concatenate([np.asarray(m[name]) for m in in_maps], axis=0)
            for name in in_names
        ]
        concat_zeros = [np.zeros(s, d) for s, d in zero_out_shapes]
        out_arrs = sharded(*concat_in, *concat_zeros)
        out_arrs = [np.asarray(o) for o in out_arrs]
        return [
            {
                name: out_arrs[i].reshape(
                    N_CORES, *out_avals[i].shape
                )[c]
                for i, name in enumerate(out_names)
            }
            for c in range(N_CORES)
        ]

    execute.in_names = in_names
    execute.out_names = out_names
    execute.out_avals = out_avals
    execute.n_params = n_params
    execute.body = _body
    execute.mesh = mesh
    execute.zero_out_shapes = zero_out_shapes
    execute.nc = nc
    _CACHE[key] = execute
    return execute


def make_in_maps(x, Wq, bq, Wk, bk, Wv, bv, Wo, bo, mode=None):
    mode = mode or DTYPE_MODE
    ne = _np_elt(mode)
    x = np.asarray(x, np.float32)
    xT = np.ascontiguousarray(x.reshape(NSEQ, D).T).astype(ne)
    in_maps = []
    for c in range(N_CORES):
        sl = slice(c * P, (c + 1) * P)
        in_maps.append(
            {
                "xT": xT,
                "wqT": np.ascontiguousarray(np.asarray(Wq)[sl, :].T).astype(ne),
                "wkT": np.ascontiguousarray(np.asarray(Wk)[sl, :].T).astype(ne),
                "wvT": np.ascontiguousarray(np.asarray(Wv)[sl, :].T).astype(ne),
                "woT": np.ascontiguousarray(np.asarray(Wo)[:, sl].T).astype(ne),
                "bqs": np.asarray(bq, np.float32)[sl].reshape(P, 1).copy(),
                "bk": np.asarray(bk, np.float32)[sl].reshape(P, 1).copy(),
                "bv": np.asarray(bv, np.float32)[sl].reshape(P, 1).copy(),
            }
        )
    return in_maps


def kernel(x, Wq, bq, Wk, bk, Wv, bv, Wo, bo):
    mode = DTYPE_MODE
    execute = _get_runner(mode)
    in_maps = make_in_maps(x, Wq, bq, Wk, bk, Wv, bv, Wo, bo, mode)
    results = execute(in_maps)
    ysum = np.zeros((B, D, S), np.float64)
    for c in range(N_CORES):
        ysum += np.asarray(results[c]["ypT"], np.float32)
    y = ysum.transpose(0, 2, 1) + np.asarray(bo, np.float32)[None, None, :]
    return np.ascontiguousarray(y.astype(np.float32))


# revision 31
# speedup vs baseline: 1.1273x; 1.1273x over previous
"""Trainium2 Bass kernel for nn_Attention_38233798869191.

Full multi-head attention layer (B=2, S=2048, D=1024, H=16, dh=64) with the
reference's "faithful to original" reshape quirk, sharded over 8 NeuronCores
by splitting heads (tensor parallel): core c owns heads {2c, 2c+1}.

Per-core dataflow (everything transposed: feature dim on SBUF partitions):
  xT [1024, 4096]     (host-pretransposed x, shared by all cores)
  qT = (WqT_c.T @ xT) * SCALE   [128, 4096]   (2 heads x 64 dims)
  kT, vT likewise.
  v natural [sk, dh] built from vT via DMA XBAR transposes (2-byte dtype),
  with an extra ones-column so the p@v matmul also produces the softmax
  denominators.
  Per (b, head): scoresT[sk, sq] = kT.T @ qT; p = exp(scoresT) (no max
  subtraction -- scores are O(1) by construction); oT' = [v | 1].T @ p
  accumulated over sk chunks in PSUM.  The accumulator (and its denominator
  row) are XBAR-transposed back to token-major, divided by the denominator
  (per-partition scalars after the transpose), and written to a DRAM
  scratch as o natural [2048, 64].
  The reference's o.reshape(B, D, S).swapaxes trick means the output
  projection is y[b].T = Wo @ M[b] where M[b][h*64:(h+1)*64] is simply
  o_natural[b,h] reinterpreted as [64, 2048] (contiguous reshape), so the
  scratch is DMA'd back as [64, 2048] rows of M. Each core computes the
  partial y[b].T = Wo[:, c*128:(c+1)*128] @ M_c[b]; partials are summed on
  the host (the tensor-parallel all-reduce) and transposed back.

Scheduling: the attention inner loop is ACT-bound (exp of [128,1024] score
tiles at ~1.25us apiece vs ~1.04us of PE work per key chunk), while the
projection / output-projection phases are PE-bound with ACT idle. Engines
consume their queues in order, so emission order IS the schedule: batch-1
projection work is chopped into ~1-2us closures injected between attention
key-chunk iterations of batch 0's heads, outproj(0) closures likewise into
batch 1's heads, and the next rep's batch-0 projection interleaves with
outproj(1) at the tail. The PE queue stays fed during exp waits and the
TensorE does nothing but 512-column matmuls.
"""

import os
import sys

import numpy as np

for _p in ("/opt/trn_rl_repo", "/root/.axon_site/_ro/trn_rl_repo"):
    if os.path.isdir(_p) and _p not in sys.path:
        sys.path.insert(0, _p)

B, S, D, H, DH = 2, 2048, 1024, 16, 64
NSEQ = B * S  # 4096
SCALE = 1.0 / float(np.sqrt(DH))
N_CORES = 8
P = 128

DTYPE_MODE = os.environ.get("KERNEL_DTYPE_MODE", "bf16")


def _build_nc(mode, reps=1):
    import concourse.bass as bass  # noqa: F401
    import concourse.mybir as mybir
    import concourse.tile as tile
    from concourse import bacc

    assert mode == "bf16", "XBAR-transpose kernel requires bf16 tiles"
    f32 = mybir.dt.float32
    elt = mybir.dt.bfloat16
    mmdt = mybir.dt.bfloat16
    AF = mybir.ActivationFunctionType

    nc = bacc.Bacc(
        "TRN2",
        target_bir_lowering=False,
        debug=False,
        num_devices=N_CORES,
    )

    xT = nc.dram_tensor("xT", [D, NSEQ], elt, kind="ExternalInput")
    wqT = nc.dram_tensor("wqT", [D, P], elt, kind="ExternalInput")
    wkT = nc.dram_tensor("wkT", [D, P], elt, kind="ExternalInput")
    wvT = nc.dram_tensor("wvT", [D, P], elt, kind="ExternalInput")
    woT = nc.dram_tensor("woT", [P, D], elt, kind="ExternalInput")
    bqs = nc.dram_tensor("bqs", [P, 1], f32, kind="ExternalInput")  # raw bq
    bk = nc.dram_tensor("bk", [P, 1], f32, kind="ExternalInput")
    bv = nc.dram_tensor("bv", [P, 1], f32, kind="ExternalInput")
    # partial y, summed across cores on the host; bf16 partials are well
    # inside the error budget (partial entries ~1e-2, quant ~0.4%)
    ypT = nc.dram_tensor("ypT", [B, D, S], elt, kind="ExternalOutput")
    osc = nc.dram_tensor("osc", [2 * 2, S, DH], elt)  # o natural per (b, hl)

    # DRAM views
    # k global = ko*512 + ks*128 + p
    xTv = xT.ap().rearrange("(ko ks p) s -> ko p ks s", ks=4, p=P)
    oscM = osc.ap().rearrange("h (r k) d -> h r (k d)", r=64)  # [4, 64, 2048]

    def wview(w):
        return w.ap().rearrange("(kc p) m -> p kc m", p=P)  # [128, 8, 128]

    with tile.TileContext(nc) as tc:
        with (
            tc.tile_pool(name="persist", bufs=1) as pp,
            tc.tile_pool(name="xin", bufs=10) as xpool,
            # PSUM budget (8 banks):
            #   sps: 2 x [128,1024] f32 = 4 banks (scores + proj + outproj)
            #   ops: 2 x [65,1024] f32  = 4 banks (o accumulator, dbl-buf)
            tc.tile_pool(name="sps", bufs=2, space="PSUM") as sps,
            tc.tile_pool(name="ops", bufs=2, space="PSUM") as ops,
            tc.tile_pool(name="ptp", bufs=4) as ptp,
            tc.tile_pool(name="oup", bufs=2) as oup,
            tc.tile_pool(name="onp", bufs=2) as onp,
            tc.tile_pool(name="vtp", bufs=2) as vtp,
            tc.tile_pool(name="rcp", bufs=3) as rcp,
            tc.tile_pool(name="obp", bufs=2) as obp,
            tc.tile_pool(name="ysb", bufs=4) as ysbp,
        ):
            # persistent SBUF tensors
            w_sb = {}
            for name, w in (("q", wqT), ("k", wkT), ("v", wvT)):
                w_sb[name] = pp.tile([P, 8, P], elt, tag=f"w{name}", name=f"w{name}")
                nc.sync.dma_start(w_sb[name][:], wview(w))
            woT_sb = pp.tile([P, D], elt, tag="wo", name="wo")
            nc.sync.dma_start(woT_sb[:], woT.ap())
            bias_sb = {}
            for name, bt in (("q", bqs), ("k", bk), ("v", bv)):
                bias_sb[name] = pp.tile([P, 1], f32, tag=f"b{name}", name=f"b{name}")
                nc.sync.dma_start(bias_sb[name][:], bt.ap())
            # ACT-path eviction computes SCALE*acc + bias, so its q bias must
            # be pre-scaled (the DVE path computes (acc + raw_bias) * SCALE)
            bias_sc = dict(bias_sb)
            bias_sc["q"] = pp.tile([P, 1], f32, tag="bqsc", name="bqsc")
            nc.vector.tensor_scalar_mul(
                bias_sc["q"][:], bias_sb["q"][:], SCALE
            )
            qT_sb = pp.tile([P, NSEQ], elt, tag="qT", name="qT")
            kT_sb = pp.tile([P, NSEQ], elt, tag="kT", name="kT")
            vT_sb = pp.tile([P, NSEQ], elt, tag="vT", name="vT")
            proj_sb = {"q": qT_sb, "k": kT_sb, "v": vT_sb}
            M_sb = [pp.tile([P, S], elt, tag=f"M{b}", name=f"M{b}") for b in range(B)]
            v_nat = [pp.tile([P, 16, 72], elt, tag=f"vn{i}", name=f"vn{i}") for i in range(4)]
            ones_sb = pp.tile([P, 16], f32, tag="ones", name="ones")
            nc.vector.memset(ones_sb[:], 1.0)
            for bh in range(4):
                # ones column survives reps: the XBAR only rewrites [:, :, 0:64]
                nc.vector.tensor_copy(v_nat[bh][:, :, 64:65], ones_sb[:, :, None])


            def proj_closures(bi, evict):
                """Batch bi's q/k/v projection + v_nat build, chopped into
                ~1-2us closures for injection into attention. `evict`
                chooses the eviction engine: DVE when injected into
                exp-bound attention (ACT saturated), ACT when emitted into
                PE-dense stretches (DVE busy with output evictions)."""
                units = []
                xtiles = {}

                def dma_unit(sq, ko):
                    def run():
                        x_sb = xpool.tile([P, 4, 512], elt, tag="x", name="x")
                        for xh in range(2):
                            nc.sync.dma_start(
                                x_sb[:, xh * 2 : (xh + 1) * 2, :],
                                xTv[
                                    ko,
                                    :,
                                    xh * 2 : (xh + 1) * 2,
                                    sq * 512 : (sq + 1) * 512,
                                ],
                            )
                        xtiles[(sq, ko)] = x_sb
                    return run

                def mm_unit(sq, n):
                    # one closure owns the PSUM accumulator's whole lifetime
                    # (8 matmuls + eviction): injected attention allocs from
                    # the same pool ring must never interleave with a live
                    # accumulator.
                    def run():
                        ps = sps.tile([P, 1024], f32, tag="s", name="s")
                        for ko in range(2):
                            for ks in range(4):
                                nc.tensor.matmul(
                                    ps[:, 0:512],
                                    w_sb[n][:, ko * 4 + ks, :].bitcast(mmdt),
                                    xtiles[(sq, ko)][:, ks, :].bitcast(mmdt),
                                    start=(ko == 0 and ks == 0),
                                    stop=(ko == 1 and ks == 3),
                                )
                        sl = slice(sq * 512, (sq + 1) * 512)
                        if evict == "dve":
                            # (acc + b) * s with per-partition bias AP
                            scale = SCALE if n == "q" else 1.0
                            nc.vector.tensor_scalar(
                                proj_sb[n][:, sl], ps[:, 0:512],
                                bias_sb[n][:], scale,
                                mybir.AluOpType.add, mybir.AluOpType.mult,
                            )
                        else:
                            kw = dict(scale=SCALE) if n == "q" else {}
                            nc.scalar.activation(
                                proj_sb[n][:, sl], ps[:, 0:512],
                                AF.Identity, bias=bias_sc[n][:], **kw,
                            )
                    return run

                def vnat_unit(hl):
                    # XBAR dst must be chunk-contiguous (strided chunk dst
                    # silently mis-strides at 16 chunks), so transpose into
                    # a contiguous temp and DVE-copy into the 72-wide
                    # ones-padded v_nat layout.
                    bh = bi * 2 + hl
                    hsl = slice(hl * 64, (hl + 1) * 64)

                    def run():
                        tmp = vtp.tile([P, 16, 64], elt, tag="vt", name="vt")
                        nc.sync.dma_start_transpose(
                            tmp[:], vT_sb[hsl, bi * S : (bi + 1) * S]
                        )
                        nc.vector.tensor_copy(v_nat[bh][:, :, 0:64], tmp[:])
                    return run

                # DMA units run ~one sq ahead of their consumers so the x
                # tiles are resident when the matmuls issue
                sqs = list(range(bi * 4, bi * 4 + 4))
                units.append(dma_unit(sqs[0], 0))
                units.append(dma_unit(sqs[0], 1))
                for i, sq in enumerate(sqs):
                    if i + 1 < len(sqs):
                        units.append(dma_unit(sqs[i + 1], 0))
                        units.append(dma_unit(sqs[i + 1], 1))
                    for n in "qkv":
                        units.append(mm_unit(sq, n))
                for hl in range(2):
                    units.append(vnat_unit(hl))
                return units

            _norm_ctr = [0]

            def normalize_half(bh, sqh, po):
                """XBAR the o accumulator (data rows + denominator row in one
                80-row tile, XBAR needs a multiple of 16) to token-major,
                divide by the per-partition denominators, store o natural to
                DRAM. No TensorE involvement."""
                obU = oup.tile([80, 1024], elt, tag="obU", name="obU")
                if _norm_ctr[0] < 2:
                    # pad rows 65..79 only once per pool slot: nothing else
                    # ever writes them, and the values are never consumed
                    nc.vector.memset(obU[64:80, :], 1.0)
                _norm_ctr[0] += 1
                nc.vector.tensor_copy(obU[0:65, :], po[:])
                onU = onp.tile([P, 8, 80], elt, tag="onU", name="onU")
                nc.sync.dma_start_transpose(onU[:], obU[:])
                rc = rcp.tile([P, 8, 1], f32, tag="rc", name="rc")
                nc.vector.reciprocal(rc[:], onU[:, :, 64:65])
                ob = obp.tile([P, 8, DH], elt, tag="ob", name="ob")
                for c in range(8):
                    nc.vector.tensor_scalar_mul(
                        ob[:, c, :], onU[:, c, 0:64], rc[:, c : c + 1, :]
                    )
                s0 = sqh * 1024
                nc.sync.dma_start(
                    osc.ap()[bh, s0 : s0 + 1024, :].rearrange(
                        "(t p) d -> p t d", p=P
                    ),
                    ob[:],
                )

            def attention_head(b, hl, extra=None, period=3):
                bh = b * 2 + hl
                hsl = slice(hl * 64, (hl + 1) * 64)
                it = 0
                for sqh in range(2):  # halves of 1024 queries
                    sq0 = b * S + sqh * 1024
                    po = ops.tile([65, 1024], f32, tag="oacc", name="oacc")

                    def emit_pv(kc, ptile, po=po, bh=bh):
                        for half in range(2):
                            nc.tensor.matmul(
                                po[:, half * 512 : (half + 1) * 512],
                                v_nat[bh][:, kc, 0:65].bitcast(mmdt),
                                ptile[
                                    :, half * 512 : (half + 1) * 512
                                ].bitcast(mmdt),
                                start=(kc == 0),
                                stop=(kc == 15),
                            )

                    # software-pipelined: pv lags two kc so the next qk runs
                    # on PE while ACT does exp
                    pending_pv = []
                    for kc in range(16):
                        k0 = b * S + kc * P
                        ps2 = sps.tile([P, 1024], f32, tag="s", name="s")
                        for half in range(2):
                            nc.tensor.matmul(
                                ps2[:, half * 512 : (half + 1) * 512],
                                kT_sb[hsl, k0 : k0 + P].bitcast(mmdt),
                                qT_sb[
                                    hsl,
                                    sq0 + half * 512 : sq0 + (half + 1) * 512,
                                ].bitcast(mmdt),
                                start=True,
                                stop=True,
                            )
                        ptile = ptp.tile([P, 1024], elt, tag="pt", name="pt")
                        nc.scalar.activation(ptile[:], ps2[:], AF.Exp)
                        pending_pv.append((kc, ptile))
                        if len(pending_pv) > 2:
                            emit_pv(*pending_pv.pop(0))
                        if extra and it % period == period - 1:
                            extra.pop(0)()
                        it += 1
                    for args in pending_pv:
                        emit_pv(*args)
                        # keep feeding the PE while the pv flush serially
                        # waits on the trailing exps
                        if extra:
                            extra.pop(0)()

                    normalize_half(bh, sqh, po)
                # M rows for this head
                nc.sync.dma_start(
                    M_sb[b][hsl.start + 0 : hsl.start + 64, :],
                    oscM[bh],
                )

            _ysb_live = {}

            def outproj_unit(b, mo, nh, evict):
                def run():
                    if nh == 0:
                        _ysb_live[(b, mo)] = ysbp.tile(
                            [P, 2 * 1024], elt, tag="y", name="y"
                        )
                    ysb = _ysb_live[(b, mo)]
                    py = sps.tile([P, 1024], f32, tag="s", name="s")
                    for half in range(2):
                        n0 = nh * 1024 + half * 512
                        nc.tensor.matmul(
                            py[:, half * 512 : (half + 1) * 512],
                            woT_sb[:, mo * P : (mo + 1) * P].bitcast(mmdt),
                            M_sb[b][:, n0 : n0 + 512].bitcast(mmdt),
                            start=True,
                            stop=True,
                        )
                    dst = ysb[:, nh * 1024 : (nh + 1) * 1024]
                    if evict == "dve" or (evict == "alt" and nh == 0):
                        nc.vector.tensor_copy(dst, py[:])
                    else:
                        nc.scalar.copy(dst, py[:])
                    if nh == 1:
                        # y writes wait on long PE->DVE chains; dispatch from
                        # GpSimd to keep them off the Sync queue
                        nc.gpsimd.dma_start(
                            ypT.ap()[b, mo * P : (mo + 1) * P, :], ysb[:]
                        )
                return run

            def outproj_units(b, evict):
                return [
                    outproj_unit(b, mo, nh, evict)
                    for mo in range(8)
                    for nh in range(2)
                ]

            def drain(units):
                while units:
                    units.pop(0)()

            # emission order is the schedule (engines dequeue in order).
            # Per rep: batch-0 attention absorbs the PREVIOUS rep's
            # outproj(1) units plus batch-1 projection closures; batch-1
            # attention absorbs outproj(0) plus the NEXT rep's batch-0
            # projection. Almost everything off the attention critical path
            # runs inside an exp-bound attention stretch, so the PE queue
            # never drains and no engine idles for long.
            # outproj units depend on M reads that clear the Sync queue a
            # few us after the producing head finishes, so they sit AFTER
            # the projection closures in each extra list — by the time the
            # popping reaches them, their M rows are resident.
            ex0 = proj_closures(0, "act")
            op1_prev = []
            for rep in range(reps):
                drain(ex0)
                exA = proj_closures(1, "dve") + op1_prev
                attention_head(0, 0, extra=exA, period=2)
                attention_head(0, 1, extra=exA, period=2)
                drain(exA)
                ex0 = proj_closures(0, "dve") if rep + 1 < reps else []
                exB = ex0 + outproj_units(0, "dve")
                attention_head(1, 0, extra=exB, period=2)
                attention_head(1, 1, extra=exB, period=2)
                drain(exB)
                ex0 = []
                op1_prev = outproj_units(1, "dve")
            drain(op1_prev)

    nc.compile()
    return nc


_CACHE = {}


def _np_elt(mode):
    import ml_dtypes

    return ml_dtypes.bfloat16


def _get_runner(mode, reps=1):
    """Build (once) the compiled kernel + a persistent jitted executor."""
    key = (mode, reps)
    if key in _CACHE:
        return _CACHE[key]

    import jax
    import jax.numpy as jnp  # noqa: F401
    from jax.sharding import Mesh, PartitionSpec
    from jax.experimental.shard_map import shard_map
    import concourse.mybir as mybir
    from concourse import bass2jax

    nc = _build_nc(mode, reps)
    bass2jax.install_neuronx_cc_hook()

    partition_name = (
        nc.partition_id_tensor.name if nc.partition_id_tensor else None
    )
    in_names = []
    out_names = []
    out_avals = []
    for alloc in nc.m.functions[0].allocations:
        if not isinstance(alloc, mybir.MemoryLocationSet):
            continue
        name = alloc.memorylocations[0].name
        if alloc.kind == "ExternalInput":
            if name != partition_name:
                in_names.append(name)
        elif alloc.kind == "ExternalOutput":
            out_names.append(name)
            shape = tuple(alloc.tensor_shape)
            dtype = mybir.dt.np(alloc.dtype)
            out_avals.append(jax.core.ShapedArray(shape, dtype))
    n_params = len(in_names)
    n_outs = len(out_avals)
    all_in_names = list(in_names) + list(out_names)
    if partition_name is not None:
        all_in_names.append(partition_name)
    all_in_names = tuple(all_in_names)

    def _body(*args):
        operands = list(args)
        if partition_name is not None:
            operands.append(bass2jax.partition_id_tensor())
        outs = bass2jax._bass_exec_p.bind(
            *operands,
            out_avals=tuple(out_avals),
            in_names=all_in_names,
            out_names=tuple(out_names),
            lowering_input_output_aliases=(),
            sim_require_finite=True,
            sim_require_nnan=True,
            nc=nc,
        )
        return tuple(outs)

    devices = jax.devices()[:N_CORES]
    mesh = Mesh(np.asarray(devices), ("core",))
    in_specs = (PartitionSpec("core"),) * (n_params + n_outs)
    out_specs = (PartitionSpec("core"),) * n_outs
    donate = tuple(range(n_params, n_params + n_outs))
    sharded = jax.jit(
        shard_map(
            _body, mesh=mesh, in_specs=in_specs, out_specs=out_specs,
            check_rep=False,
        ),
        donate_argnums=donate,
        keep_unused=True,
    )

    zero_out_shapes = [
        ((N_CORES * a.shape[0],) + tuple(a.shape[1:]), a.dtype)
        for a in out_avals
    ]

    def execute(in_maps):
        concat_in = [
            np.concatenate([np.asarray(m[name]) for m in in_maps], axis=0)
            for name in in_names
        ]
        concat_zeros = [np.zeros(s, d) for s, d in zero_out_shapes]
        out_arrs = sharded(*concat_in, *concat_zeros)
        out_arrs = [np.asarray(o) for o in out_arrs]
        return [
            {
                name: out_arrs[i].reshape(
                    N_CORES, *out_avals[i].shape
                )[c]
                for i, name in enumerate(out_names)
            }
            for c in range(N_CORES)
        ]

    execute.in_names = in_names
    execute.out_names = out_names
    execute.out_avals = out_avals
    execute.n_params = n_params
    execute.body = _body
    execute.mesh = mesh
    execute.zero_out_shapes = zero_out_shapes
    execute.nc = nc
    _CACHE[key] = execute
    return execute


def make_in_maps(x, Wq, bq, Wk, bk, Wv, bv, Wo, bo, mode=None):
    mode = mode or DTYPE_MODE
    ne = _np_elt(mode)
    x = np.asarray(x, np.float32)
    xT = np.ascontiguousarray(x.reshape(NSEQ, D).T).astype(ne)
    in_maps = []
    for c in range(N_CORES):
        sl = slice(c * P, (c + 1) * P)
        in_maps.append(
            {
                "xT": xT,
                "wqT": np.ascontiguousarray(np.asarray(Wq)[sl, :].T).astype(ne),
                "wkT": np.ascontiguousarray(np.asarray(Wk)[sl, :].T).astype(ne),
                "wvT": np.ascontiguousarray(np.asarray(Wv)[sl, :].T).astype(ne),
                "woT": np.ascontiguousarray(np.asarray(Wo)[:, sl].T).astype(ne),
                "bqs": np.asarray(bq, np.float32)[sl].reshape(P, 1).copy(),
                "bk": np.asarray(bk, np.float32)[sl].reshape(P, 1).copy(),
                "bv": np.asarray(bv, np.float32)[sl].reshape(P, 1).copy(),
            }
        )
    return in_maps


def kernel(x, Wq, bq, Wk, bk, Wv, bv, Wo, bo):
    mode = DTYPE_MODE
    execute = _get_runner(mode)
    in_maps = make_in_maps(x, Wq, bq, Wk, bk, Wv, bv, Wo, bo, mode)
    results = execute(in_maps)
    ysum = np.zeros((B, D, S), np.float64)
    for c in range(N_CORES):
        ysum += np.asarray(results[c]["ypT"], np.float32)
    y = ysum.transpose(0, 2, 1) + np.asarray(bo, np.float32)[None, None, :]
    return np.ascontiguousarray(y.astype(np.float32))
